# revision 47
# baseline (speedup 1.0000x reference)
"""Trainium2 Bass kernel for BertSelfAttention + LoRA (bs=4, seq=2048, hidden=1024, 16 heads).

Sharding: 8 cores = 4 batches x 2 head-groups. Each core handles one batch and 8
heads (512 of the 1024 hidden output dims). LoRA is folded into the weights on
the host (W_eff = W + scaling * B @ A  — algebraically identical), and x / W_eff
are pre-cast to bf16 on the host.

Per-core device kernel (all matmuls bf16, accumulation fp32 in PSUM):
  x16 [2048,1024] bf16  --xbar transpose DMA-->  xT [1024,2048] in SBUF
  W slices likewise -> wT [1024,512] per projection.
  QK^T projections computed transposed:  qT/kT [d'=512, tok] (bias folded into
  the PSUM->SBUF cast via per-partition tensor_scalar_add).
  V computed in natural [tok, d'] layout (bias via a K=1 ones-row matmul pass),
  assembled into per-token-tile V' tiles [128, 8*65] with a ones column per head.
  Attention per head PAIR (row-packed: head0 on array rows 0-63, head1 on rows
  64-127 -> concurrent PE row groups on HW), fully transposed:
    scoresT[k,q] = K^T(d,k).T @ Q^T(d,q)      (PSUM [128,1024])
    expT = exp(scoresT/8 + mask[k])           (ACT, mask = per-partition bias)
    outT[d',q] += V'[k,d'].T @ expT[k,q]      (PSUM [65,512]; row 64 = denom)
  head0's PV runs inline; head1's exp tiles persist and its PV runs as a second
  pass, so only 2 PV PSUM banks are live at a time (total 8 banks exactly).
  Projection work (V for group 0, QK of group g+1 during group g) is emitted
  interleaved into the attention kt-loop so the PE fills the ACT-bound gaps.
  Host divides by the denominator row and transposes during the gather.

  Engine balance: the even head's exp runs on ACT (exact spline exp); the odd
  head's exp runs on DVE via a Schraudolph bit-trick (t = s*16*log2e + magic,
  f32->int16 round on the tensor_scalar write, bits reinterpreted as bf16 =
  2^y with linear mantissa, ~1.8% rms).  The systematic part of the trick's
  error is common to a whole head's softmax rows and cancels in the num/den
  ratio.  PV output copies run on ACT (Copy shares the exp table set), so the
  per-kt steady state is PE-bound with ACT/DVE each below it.
"""

import math
import numpy as np

BS = 4
SEQ = 2048
HID = 1024
HEADS = 16
HD = 64
RANK = 16
LORA_SCALING = 1.0 / RANK

N_CORES = 8
NH = 8          # heads per core
DP = 512        # output dims per core (NH * HD)
P = 128
NT = SEQ // P   # 16 token tiles
NHB = HID // P  # 8 hidden blocks
NG = DP // P    # 4 d' groups (2 heads each)
VW = HD + 1     # 65: V columns + ones column
VS = 128        # V' stationary stride per head, padded to 128 so the PV
                # matmuls keep a 128-column stationary -> FWL stays enabled
                # (a 65-col stationary forces slow LDWEIGHTS, ~+80ns/MM)

# Schraudolph exp-to-bf16 on DVE: t16 = s*(16*log2e) + (mask*128*log2e + 16256-C)
SCH_A = 16.0 * math.log2(math.e)
SCH_MASK = 128.0 * math.log2(math.e)
SCH_C = 7.0          # centers the (1+f) vs 2^f mantissa error (rms ~1.8%)

_CACHE = {}


def _build():
    import concourse.bass as bass
    import concourse.tile as tile
    from concourse import bacc, mybir

    f32 = mybir.dt.float32
    bf16 = mybir.dt.bfloat16
    i16 = mybir.dt.int16
    Exp = mybir.ActivationFunctionType.Exp
    Ident = mybir.ActivationFunctionType.Identity
    Mult = mybir.AluOpType.mult
    Add = mybir.AluOpType.add

    nc = bacc.Bacc("TRN2", target_bir_lowering=False, debug=False,
                   num_devices=N_CORES)

    xTin = nc.dram_tensor("xT16", [HID, SEQ], bf16, kind="ExternalInput").ap()
    wTin = [nc.dram_tensor(f"wT{n}", [HID, DP], bf16, kind="ExternalInput").ap()
            for n in "qkv"]
    bias_qk = [nc.dram_tensor(f"b{n}", [P, NG], f32, kind="ExternalInput").ap()
               for n in "qk"]
    mask = nc.dram_tensor("mask", [P, NT], f32, kind="ExternalInput").ap()
    dmask = nc.dram_tensor("dmask", [P, NT], f32, kind="ExternalInput").ap()
    out = nc.dram_tensor("out", [NH, VW, SEQ], f32, kind="ExternalOutput").ap()

    with tile.TileContext(nc) as tc:
        with (
            tc.tile_pool(name="consts", bufs=1) as cpool,
            tc.tile_pool(name="xT", bufs=1) as xT_pool,
            tc.tile_pool(name="wT", bufs=1) as wT_pool,
            tc.tile_pool(name="qkT", bufs=1) as qkT_pool,
            tc.tile_pool(name="vp", bufs=1) as vp_pool,
            tc.tile_pool(name="expp", bufs=8) as exp_pool,
            tc.tile_pool(name="exp1", bufs=18) as exp1_pool,
            tc.tile_pool(name="outp", bufs=6) as out_pool,
            tc.tile_pool(name="ps_sc0", bufs=2, space="PSUM") as ps_sc0,
            tc.tile_pool(name="ps_sc1", bufs=2, space="PSUM") as ps_sc1,
            tc.tile_pool(name="ps_proj", bufs=2, space="PSUM") as ps_proj,
            tc.tile_pool(name="ps_pv", bufs=2, space="PSUM") as ps_pv,
        ):
            # PSUM: four dedicated pools of 2 one-bank [128,512] slots each
            # (8 banks total).  Scores land in per-qh half tiles; each
            # half-exp (ACT for the even head, DVE Schraudolph for the odd)
            # frees its bank in ~0.7us, so the exp->next-kt-scores recycling
            # chain stays under the PE period and nothing shares a ring with
            # the projection scratch (whose cast can sit in an engine queue).
            # ---- constants ----
            mask_t = cpool.tile([P, NT], f32, tag="mask", name="mask_t")
            nc.sync.dma_start(mask_t[:], mask[:, :])
            dmask_t = cpool.tile([P, NT], f32, tag="dmask", name="dmask_t")
            nc.sync.dma_start(dmask_t[:], dmask[:, :])
            bias_t = []
            for i in range(2):
                bt = cpool.tile([P, NG], f32, tag=f"bias{i}", name=f"bias{i}")
                nc.sync.dma_start(bt[:], bias_qk[i][:, :])
                bias_t.append(bt)

            # ---- plain contiguous loads (x and W arrive pre-transposed) ----
            # One DMA per W: all 8 hid-chunks land in one [128, 8*DP] tile
            # (chunk c at cols c*DP), so SP pays one descriptor setup instead
            # of eight.  x loads in two half-seq DMAs per hid-chunk tile.
            wTa = [wT_pool.tile([P, NHB * DP], bf16, tag=f"wTa{w}",
                                name=f"wTa{w}") for w in range(3)]
            wT = [[wTa[w][:, c * DP:(c + 1) * DP] for c in range(NHB)]
                  for w in range(3)]
            xT = [xT_pool.tile([P, SEQ], bf16, tag=f"xT{c}", name=f"xT{c}")
                  for c in range(NHB)]

            def load_w(w):
                src = wTin[w].rearrange("(c p) d -> p c d", p=P)
                dst = wTa[w][:].rearrange("p (c d) -> p c d", d=DP)
                nc.sync.dma_start(dst, src)

            def load_x(half, c):
                r = slice(half * 1024, (half + 1) * 1024)
                nc.sync.dma_start(xT[c][:, r], xTin[c * P:(c + 1) * P, r])

            # PE warm-up burst: ~3.5us of tiny matmuls during the initial
            # DMA window releases the HAM clock gate before real work lands
            warm = cpool.tile([64, 128], bf16, tag="warm", name="warm")
            nc.gpsimd.memset(warm[:], 0.0078125)
            wps = ps_proj.tile([64, 128], f32, tag="proj", name="wps")
            for i in range(36):
                nc.tensor.matmul(wps[:], warm[:, 0:64], warm[:],
                                 start=(i == 0), stop=(i == 35))
            wsink = cpool.tile([64, 128], f32, tag="wsink", name="wsink")
            nc.vector.tensor_copy(wsink[:], wps[:])

            # K-weights + x first half first (they gate the first QK units),
            # then Q, V weights, then the back half of x
            load_w(1)
            for c in range(NHB):
                load_x(0, c)
            load_w(0)
            load_w(2)
            for c in range(NHB):
                load_x(1, c)

            qkT = [[qkT_pool.tile([P, SEQ], bf16, tag=f"qkT{w}_{g}",
                                  name=f"qkT{w}_{g}")
                    for g in range(NG)] for w in range(2)]
            vp = [vp_pool.tile([P, NH * VS], bf16, tag=f"vp{tt}",
                               name=f"vp{tt}") for tt in range(NT)]
            # one-time V' init: zero the pad columns, ones in column 64 of
            # each head's 128-col block (V-copies never touch either)
            for tt in range(NT):
                nc.gpsimd.memset(vp[tt][:], 0.0)
                ones_col = vp[tt][:].rearrange("p (h c) -> p h c",
                                               c=VS)[:, :, HD:HD + 1]
                nc.gpsimd.memset(ones_col, 1.0)

            def emit_v_unit(tt):
                """V projection for token tile tt + V' assembly."""
                ps = ps_proj.tile([P, DP], f32, tag="proj", name="ps_v")
                for p in range(NHB):
                    nc.tensor.matmul(ps[:],
                                     xT[p][:, tt * P:(tt + 1) * P],
                                     wT[2][p][:],
                                     start=(p == 0), stop=(p == NHB - 1))
                dst = vp[tt][:].rearrange("p (h c) -> p h c", c=VS)[:, :, 0:HD]
                nc.scalar.copy(dst,
                               ps[:].rearrange("p (h c) -> p h c", c=HD))

            def emit_qk_unit(g, w, tc):
                """Q^T/K^T projection for group g, proj w, tokchunk tc:
                8 accumulating matmuls + the bias/cast to SBUF bf16."""
                gs = slice(g * P, (g + 1) * P)
                ps = ps_proj.tile([P, 512], f32, tag="proj", name="ps_qk")
                for p in range(NHB):
                    nc.tensor.matmul(ps[:], wT[w][p][:, gs],
                                     xT[p][:, tc * 512:(tc + 1) * 512],
                                     start=(p == 0), stop=(p == NHB - 1))
                nc.scalar.activation(
                    qkT[w][g][:, tc * 512:(tc + 1) * 512],
                    ps[:], Ident,
                    bias=bias_t[w][:, g:g + 1])

            # upfront: just enough projection for the first scores + exps
            emit_qk_unit(0, 1, 0)   # K^T group0 tok 0:512 (covers kt 0..3)
            emit_qk_unit(0, 0, 0)   # Q^T group0 tok 0:512
            emit_qk_unit(0, 0, 1)   # Q^T group0 tok 512:1024
            for tt in range(4):
                emit_v_unit(tt)

            # per-(g,qb) filler schedules: step -> [closures]
            def sched_of(g, qb):
                s = {}

                def put(step, *cl):
                    s.setdefault(step, []).extend(cl)

                if (g, qb) == (0, 0):
                    for tt in range(4, NT):
                        put(tt - 4, lambda tt=tt: emit_v_unit(tt))
                    put(0, lambda: emit_qk_unit(0, 1, 1))   # K tc1 (kt 4..7)
                    put(2, lambda: emit_qk_unit(0, 1, 2))   # K tc2 (kt 8..11)
                    put(4, lambda: emit_qk_unit(0, 1, 3))   # K tc3
                    put(6, lambda: emit_qk_unit(0, 0, 2))   # Q tc2 (qb1)
                    put(8, lambda: emit_qk_unit(0, 0, 3))   # Q tc3
                elif (g, qb) == (0, 1):
                    units = [(w, tc) for w in (0, 1) for tc in range(4)]
                    for i, (w, tc) in enumerate(units):
                        put(4 + i, lambda w=w, tc=tc:
                            emit_qk_unit(1, w, tc))
                elif qb == 0 and 0 < g < NG - 1:
                    for tc in range(4):              # Q of g+1
                        put(8 + 2 * tc, lambda tc=tc:
                            emit_qk_unit(g + 1, 0, tc))
                elif qb == 1 and 0 < g < NG - 1:
                    for tc in range(4):              # K of g+1
                        put(4 + 2 * tc, lambda tc=tc:
                            emit_qk_unit(g + 1, 1, tc))
                return s

            pending = None          # (h1, qb, et1 tiles) awaiting PV pass

            def emit_pvh1_chunk(pend, j, pvt1):
                h1p, qbp, et1p = pend
                vb1 = h1p * VS
                for kt in range(4 * j, 4 * j + 4):
                    for qc in range(2):
                        nc.tensor.matmul(pvt1[qc][:],
                                         vp[kt][:, vb1:vb1 + VS],
                                         et1p[kt][:, qc * 512:(qc + 1) * 512],
                                         start=(kt == 0),
                                         stop=(kt == NT - 1))

            def emit_pvh1_outs(pend, pvt1):
                h1p, qbp, _ = pend
                for qc in range(2):
                    ot = out_pool.tile([VW, 512], f32, tag="ot", name="ot")
                    nc.vector.tensor_copy(ot[:], pvt1[qc][0:VW, :])
                    q0 = qbp * 1024 + qc * 512
                    nc.sync.dma_start(out[h1p][:, q0:q0 + 512], ot[:])

            for g in range(NG):
                h0, h1 = 2 * g, 2 * g + 1
                sl0, sl1 = slice(0, HD), slice(HD, P)
                for qb in range(2):
                    sched = sched_of(g, qb)
                    et0s = []
                    et1 = []
                    pvt0 = None
                    pvt1_prev = None
                    for kt in range(NT):
                        if kt == 0 and pending is not None:
                            pvt1_prev = [ps_pv.tile([P, 512], f32, tag="pv",
                                                    name="pv1")
                                         for _ in range(2)]
                        if kt < 4 and pending is not None:
                            emit_pvh1_chunk(pending, kt, pvt1_prev)
                            if kt == 3:
                                emit_pvh1_outs(pending, pvt1_prev)
                                pending = None
                        for f in sched.get(kt, ()):
                            f()
                        ks = slice(kt * P, (kt + 1) * P)
                        sc0h = [ps_sc0.tile([P, 512], f32, tag="sc0",
                                            name="sc0")
                                for _ in range(2)]
                        sc1h = [ps_sc1.tile([P, 512], f32, tag="sc1",
                                            name="sc1")
                                for _ in range(2)]
                        for qh in range(2):
                            q0 = qb * 1024 + qh * 512
                            qs = slice(q0, q0 + 512)
                            nc.tensor.matmul(sc0h[qh][:], qkT[1][g][sl0, ks],
                                             qkT[0][g][sl0, qs],
                                             start=True, stop=True)
                            nc.tensor.matmul(sc1h[qh][:], qkT[1][g][sl1, ks],
                                             qkT[0][g][sl1, qs],
                                             start=True, stop=True)
                        et0 = exp_pool.tile([P, 1024], bf16, tag="exp",
                                            name="et0")
                        for qh in range(2):
                            os_ = slice(qh * 512, (qh + 1) * 512)
                            nc.scalar.activation(et0[:, os_], sc0h[qh][:],
                                                 Exp,
                                                 bias=mask_t[:, kt:kt + 1],
                                                 scale=0.125)
                        et0s.append(et0)
                        et1k = exp1_pool.tile([P, 1024], bf16, tag="exp1",
                                              name="et1")
                        for qh in range(2):
                            os_ = slice(qh * 512, (qh + 1) * 512)
                            nc.vector.tensor_scalar(
                                et1k[:, os_].bitcast(i16), sc1h[qh][:],
                                SCH_A, dmask_t[:, kt:kt + 1], Mult, Add)
                        et1.append(et1k)
                        if kt == 4:
                            pvt0 = [ps_pv.tile([P, 512], f32, tag="pv",
                                               name="pv0") for _ in range(2)]
                        if kt >= 4:
                            ktl = kt - 4   # lagged PV for head0
                            vb0 = h0 * VS
                            for qc in range(2):
                                nc.tensor.matmul(
                                    pvt0[qc][:],
                                    vp[ktl][:, vb0:vb0 + VS],
                                    et0s[ktl][:, qc * 512:(qc + 1) * 512],
                                    start=(ktl == 0), stop=False)
                    for kt in range(NT - 4, NT):
                        vb0 = h0 * VS
                        for qc in range(2):
                            nc.tensor.matmul(pvt0[qc][:],
                                             vp[kt][:, vb0:vb0 + VS],
                                             et0s[kt][:, qc * 512:(qc + 1) * 512],
                                             start=False, stop=(kt == NT - 1))
                    for qc in range(2):
                        ot = out_pool.tile([VW, 512], f32, tag="ot", name="ot")
                        nc.vector.tensor_copy(ot[:], pvt0[qc][0:VW, :])
                        q0 = qb * 1024 + qc * 512
                        nc.sync.dma_start(out[h0][:, q0:q0 + 512], ot[:])
                    pending = (h1, qb, et1)

            # tail: the last block's odd head runs as a final pending pass
            pvt1_prev = [ps_pv.tile([P, 512], f32, tag="pv", name="pv1")
                         for _ in range(2)]
            for j in range(4):
                emit_pvh1_chunk(pending, j, pvt1_prev)
            emit_pvh1_outs(pending, pvt1_prev)
            pending = None

    nc.compile()
    return nc


def _get_nc():
    if "nc" not in _CACHE:
        _CACHE["nc"] = _build()
    return _CACHE["nc"]


def kernel(hidden_states, attention_mask, Wq, bq, Aq, Bq, Wk, bk, Ak, Bk,
           Wv, bv, Av, Bv):
    from concourse import bass_utils
    import ml_dtypes
    import os

    nc = _get_nc()
    bf = ml_dtypes.bfloat16

    hs = np.asarray(hidden_states, dtype=np.float32)
    am = np.asarray(attention_mask, dtype=np.float32)
    weff = {}
    for n, W, A, B in (("q", Wq, Aq, Bq), ("k", Wk, Ak, Bk), ("v", Wv, Av, Bv)):
        W = np.asarray(W, dtype=np.float32)
        A = np.asarray(A, dtype=np.float32)
        B = np.asarray(B, dtype=np.float32)
        weff[n] = (W + LORA_SCALING * (B @ A)).astype(bf)
    biases = {"q": np.asarray(bq, np.float32), "k": np.asarray(bk, np.float32),
              "v": np.asarray(bv, np.float32)}
    hs16T = [np.ascontiguousarray(hs[b].T.astype(bf)) for b in range(BS)]

    in_maps = []
    for c in range(N_CORES):
        b, hg = divmod(c, 2)
        rows = slice(hg * DP, (hg + 1) * DP)
        mcols = np.ascontiguousarray(am[b, 0, 0].reshape(NT, P).T)
        m = {
            "xT16": hs16T[b],
            "mask": mcols,
            "dmask": (mcols * SCH_MASK + (16256.0 - SCH_C)).astype(np.float32),
        }
        for n in ("q", "k", "v"):
            m[f"wT{n}"] = np.ascontiguousarray(weff[n][rows].T)
        for n in ("q", "k"):
            m[f"b{n}"] = np.ascontiguousarray(
                biases[n][rows].reshape(NG, P).T)
        in_maps.append(m)

    trace = bool(int(os.environ.get("BASS_KERNEL_TRACE", "0")))
    res = bass_utils.run_bass_kernel_spmd(nc, in_maps,
                                          core_ids=list(range(N_CORES)),
                                          trace=trace)
    _CACHE["last_results"] = res

    output = np.empty((BS, SEQ, HID), dtype=np.float32)
    for c in range(N_CORES):
        b, hg = divmod(c, 2)
        r = res.results[c]["out"]                      # [NH, 65, SEQ]
        o = r[:, :HD, :] / r[:, HD:HD + 1, :]          # [NH, 64, SEQ]
        rows = slice(hg * DP, (hg + 1) * DP)
        output[b, :, rows] = (o.transpose(2, 0, 1).reshape(SEQ, DP)
                              + biases["v"][rows][None, :])
    return output

